# revision 1
# baseline (speedup 1.0000x reference)
"""CollaborativeAttention (complex-valued, per-head mixed queries) on 8 trn2 cores.

Sharding: B*H = 24 (batch, head) units -> 3 heads per core.
  core c: batch b = c // 4, head block hb = c % 4 -> heads [3*hb, 3*hb+2].
Each core computes q/k projections for its batch (replicated within the
4-core batch group), v/cb projections for its head block only, then
scores+softmax+context for its 3 heads.

Projections and scores run as float32r (full PE rate at >=256 moving rows,
better precision than bf16); the small-moving-dim (129) context matmul runs
in fp16 (10-bit mantissa, still 1 cyc/row; fp32r would take a 4x penalty
below 256 moving rows, and bf16 costs ~2e-3 relative error). PSUM accumulation is fp32.
This walrus build encodes at most one sync-wait per instruction, so a
post-pass (_split_multi_waits) peels extra waits onto NoOps.

Layout notes:
 - hidden is transposed on the HOST (fp32 has no DMA transpose); we ship
   hT[d, s] plus a pre-negated imaginary copy for the complex-linear subtract.
 - scores are computed transposed, sT[t, s] (t = key pos on partitions), so
   the content bias (indexed by t) is a per-partition ACT bias fused into the
   exp, and so probs land directly in the lhsT layout the context matmul wants.
 - softmax denominators come from a ones-column appended to [vr | vi] in the
   context matmul's moving operand; normalization happens on the tiny
   [128, 64] context tiles instead of the [1024, 1024] prob matrices.
"""

import sys

for _p in ("/opt/trn_rl_repo", "/root/.axon_site", "/root/.axon_site/_ro/trn_rl_repo",
           "/root/.axon_site/_ro/pypackages"):
    if _p not in sys.path:
        sys.path.append(_p)

import numpy as np

import concourse.bass as bass
import concourse.mybir as mybir
import concourse.tile as tile
from concourse.bass_utils import run_bass_kernel_spmd

B, S, D, H = 2, 1024, 768, 12
DK = DV = 768
DH = DV // H          # 64 per-head value dim
HPC = 3               # heads per core
N_CORES = 8
P = 128
ND = D // P           # 6 d-tiles (contraction)
NT = S // P           # 8 token tiles
SW = 512              # s-slice width for scores/projections
NS = S // SW          # 2 s-slices
VC = HPC * DH         # 192 value cols per core
WVCB = 2 * VC + 2 * HPC  # 390: [Wv_r | Wv_i | Wcb_r | Wcb_i] cols

FP = mybir.dt.float32
FR = mybir.dt.float32r
BF = mybir.dt.bfloat16
HF = mybir.dt.float16
AF = mybir.ActivationFunctionType
OP = mybir.AluOpType

TRACE = False
LAST_RESULTS = None

_compiled = None


def _split_multi_waits(nc):
    """The walrus build here encodes at most ONE sync-wait per instruction
    ("Too many sync wait commands" in setupSyncWait otherwise). Tile freely
    emits several. Split the extras onto single-wait NoOps that precede the
    instruction in the same engine stream."""
    for fn in nc.m.functions:
        for bb in fn.blocks:
            out = []
            for ins in bb.instructions:
                si = ins.sync_info
                if si is not None and len(si.on_wait) > 1:
                    waits = list(si.on_wait)
                    for j, w in enumerate(waits[:-1]):
                        nop = mybir.InstNoOp(name=f"{ins.name}-ws{j}",
                                             ins=[], outs=[])
                        nop.engine = ins.engine
                        nop.sync_info = mybir.SyncInfo(on_wait=[w], on_update=[])
                        out.append(nop)
                    ins.sync_info = mybir.SyncInfo(on_wait=[waits[-1]],
                                                   on_update=list(si.on_update))
                out.append(ins)
            bb.instructions = out


def _build():
    """Build the SPMD Bass program (identical on all 8 cores)."""
    nc = bass.Bass(trn_type="TRN2")

    hTr_d = nc.dram_tensor("hTr", [ND, P, S], FR, kind="ExternalInput")
    hTi_d = nc.dram_tensor("hTi", [ND, P, S], FR, kind="ExternalInput")
    hTin_d = nc.dram_tensor("hTin", [ND, P, S], FR, kind="ExternalInput")
    wqr_d = nc.dram_tensor("wqr", [ND, P, DK], FR, kind="ExternalInput")
    wqi_d = nc.dram_tensor("wqi", [ND, P, DK], FR, kind="ExternalInput")
    wkr_d = nc.dram_tensor("wkr", [ND, P, DK], FR, kind="ExternalInput")
    wki_d = nc.dram_tensor("wki", [ND, P, DK], FR, kind="ExternalInput")
    wvcb_d = nc.dram_tensor("wvcb", [ND, P, WVCB], FR, kind="ExternalInput")
    bvec_d = nc.dram_tensor("bvec", [1, P + WVCB], FR, kind="ExternalInput")
    mixv_d = nc.dram_tensor("mixv", [HPC, ND, 3, P], FP, kind="ExternalInput")
    out_d = nc.dram_tensor("out", [2, HPC, NT, P, DH], FP, kind="ExternalOutput")

    NDK = DK // P  # 6 output n-tiles for q/k

    with tile.TileContext(nc) as tc:
        with (
            tc.tile_pool(name="persist", bufs=1) as persist,
            tc.tile_pool(name="vstuff", bufs=1) as vstuff,
        ):
            # ---- persistent tensors -------------------------------------
            qTr = persist.tile([P, NDK, S], FP)
            qTi = persist.tile([P, NDK, S], FP)
            kTr = persist.tile([P, NDK, S], FR)
            kTi = persist.tile([P, NDK, S], FR)

            wvcb_sb = vstuff.tile([P, ND, WVCB], FR)
            nc.sync.dma_start(wvcb_sb, wvcb_d[:].rearrange("a p c -> p a c"))
            # [ones(P) | bv_r | bv_i | 0]: ones for the rank-1 bias matmul
            bvec_sb = vstuff.tile([1, P + WVCB], FR)
            nc.sync.dma_start(bvec_sb, bvec_d[:])
            mixv = vstuff.tile([P, HPC * ND * 3], FP)
            nc.sync.dma_start(
                mixv, mixv_d[:].rearrange("h a c p -> p (h a c)"))
            # per-head context rhs: [vr_h | vi_h | 1]
            vaug = [vstuff.tile([P, NT, 2 * DH + 1], HF, tag=f"vaug{h}",
                                name=f"vaug{h}")
                    for h in range(HPC)]
            for h in range(HPC):
                nc.vector.memset(vaug[h][:, :, 2 * DH], 1.0)
            # (cbr/8 | cbi/8) per head, flattened: col = tt*2*HPC + (0|HPC) + h
            cb8 = vstuff.tile([P, NT * 2 * HPC], FP)

            # ---- phase P: projections -----------------------------------
            with (
                tc.tile_pool(name="hload", bufs=1) as hload,
                tc.tile_pool(name="wstream", bufs=4) as wstream,
                tc.tile_pool(name="pproj", bufs=1, space="PSUM") as pproj,
                tc.tile_pool(name="pv", bufs=1, space="PSUM") as pv,
                tc.tile_pool(name="cbtmp", bufs=2) as cbtmp,
                tc.tile_pool(name="vstage", bufs=2) as vstage,
            ):
                for ss in range(NS):
                    ssl = slice(ss * SW, (ss + 1) * SW)
                    hr = hload.tile([P, ND, SW], FR, tag="hr")
                    hi = hload.tile([P, ND, SW], FR, tag="hi")
                    hin = hload.tile([P, ND, SW], FR, tag="hin")
                    nc.sync.dma_start(hr, hTr_d[:, :, ssl].rearrange("a p s -> p a s"))
                    nc.sync.dma_start(hi, hTi_d[:, :, ssl].rearrange("a p s -> p a s"))
                    nc.sync.dma_start(hin, hTin_d[:, :, ssl].rearrange("a p s -> p a s"))

                    # q/k projections: out[n, s] = sum_d W[d, n] * hT[d, s]
                    for (wr_d, wi_d, dst_r, dst_i, gname) in (
                        (wqr_d, wqi_d, qTr, qTi, "q"),
                        (wkr_d, wki_d, kTr, kTi, "k"),
                    ):
                        for ntb in range(2):  # n-tile blocks of 3
                            nts = range(3 * ntb, 3 * ntb + 3)
                            psr = {nt: pproj.tile([P, SW], FP, tag=f"psr{nt % 3}",
                                                  name=f"psr{nt}")
                                   for nt in nts}
                            psi = {nt: pproj.tile([P, SW], FP, tag=f"psi{nt % 3}",
                                                  name=f"psi{nt}")
                                   for nt in nts}
                            for d in range(ND):
                                wr = wstream.tile([P, DK], FR, tag="wr")
                                wi = wstream.tile([P, DK], FR, tag="wi")
                                nc.sync.dma_start(wr, wr_d[d])
                                nc.sync.dma_start(wi, wi_d[d])
                                for nt in nts:
                                    nsl = slice(nt * P, (nt + 1) * P)
                                    st, sp = d == 0, d == ND - 1
                                    # real: Wr.hr + Wi.(-hi)
                                    nc.tensor.matmul(psr[nt], wr[:, nsl],
                                                     hr[:, d], start=st, stop=False)
                                    nc.tensor.matmul(psr[nt], wi[:, nsl],
                                                     hin[:, d], start=False, stop=sp)
                                    # imag: Wi.hr + Wr.hi
                                    nc.tensor.matmul(psi[nt], wi[:, nsl],
                                                     hr[:, d], start=st, stop=False)
                                    nc.tensor.matmul(psi[nt], wr[:, nsl],
                                                     hi[:, d], start=False, stop=sp)
                            for nt in nts:
                                nc.scalar.activation(dst_r[:, nt, ssl], psr[nt], AF.Copy)
                                nc.scalar.activation(dst_i[:, nt, ssl], psi[nt], AF.Copy)

                    # v / cb projections: [tok, c] = sum_d hT[d, tok] * Wbig[d, c]
                    for tt in range(ss * NT // NS, (ss + 1) * NT // NS):
                        tsl = slice((tt * P) % SW, (tt * P) % SW + P)
                        psA = pv.tile([P, WVCB], FP, tag="psA")
                        psB = pv.tile([P, WVCB], FP, tag="psB")
                        # bias row (bv | 0) into psA first (fewest deps first)
                        nc.tensor.matmul(psA, bvec_sb[:, :P], bvec_sb[:, P:],
                                         start=True, stop=False)
                        for d in range(ND):
                            nc.tensor.matmul(psA, hr[:, d, tsl], wvcb_sb[:, d],
                                             start=False, stop=(d == ND - 1))
                            nc.tensor.matmul(psB, hi[:, d, tsl], wvcb_sb[:, d],
                                             start=(d == 0), stop=(d == ND - 1))
                        # DVE can read only one PSUM operand; stage A in SBUF
                        sA = vstage.tile([P, WVCB], FP, tag="sA")
                        nc.scalar.activation(sA, psA, AF.Copy)
                        psA = sA
                        for h in range(HPC):
                            c0 = h * DH
                            # vr_h = A[vr] - B[vi];  vi_h = A[vi] + B[vr]
                            nc.vector.tensor_sub(vaug[h][:, tt, 0:DH],
                                                 psA[:, c0:c0 + DH],
                                                 psB[:, VC + c0:VC + c0 + DH])
                            nc.vector.tensor_add(vaug[h][:, tt, DH:2 * DH],
                                                 psA[:, VC + c0:VC + c0 + DH],
                                                 psB[:, c0:c0 + DH])
                        # cb8: (A[cbr] - B[cbi])/8 , (A[cbi] + B[cbr])/8
                        tr = cbtmp.tile([P, HPC], FP, tag="tr")
                        ti = cbtmp.tile([P, HPC], FP, tag="ti")
                        nc.vector.tensor_sub(tr, psA[:, 2 * VC:2 * VC + HPC],
                                             psB[:, 2 * VC + HPC:2 * VC + 2 * HPC])
                        nc.vector.tensor_add(ti, psA[:, 2 * VC + HPC:2 * VC + 2 * HPC],
                                             psB[:, 2 * VC:2 * VC + HPC])
                        cbc = tt * 2 * HPC
                        nc.vector.tensor_scalar_mul(cb8[:, cbc:cbc + HPC], tr, 0.125)
                        nc.vector.tensor_scalar_mul(
                            cb8[:, cbc + HPC:cbc + 2 * HPC], ti, 0.125)

            # ---- phase S: per-head scores -> softmax -> context ---------
            with (
                tc.tile_pool(name="mqp", bufs=1) as mqp,
                tc.tile_pool(name="ep", bufs=1) as ep,
                tc.tile_pool(name="psc", bufs=2, space="PSUM") as psc,
                tc.tile_pool(name="pctx", bufs=2, space="PSUM") as pctx,
                tc.tile_pool(name="ctxs", bufs=4) as ctxs,
            ):
                for h in range(HPC):
                    for ss in range(NS):
                        ssl = slice(ss * SW, (ss + 1) * SW)
                        mqr = mqp.tile([P, NDK, SW], FR, tag="mqr")
                        mqi = mqp.tile([P, NDK, SW], FR, tag="mqi")
                        mqin = mqp.tile([P, NDK, SW], FR, tag="mqin")
                        for a in range(NDK):
                            mbase = (h * ND + a) * 3
                            mr = mixv[:, mbase:mbase + 1]
                            mi = mixv[:, mbase + 1:mbase + 2]
                            min_ = mixv[:, mbase + 2:mbase + 3]
                            # mqr = qTr*mr - qTi*mi ; mqi = qTr*mi + qTi*mr
                            nc.vector.tensor_scalar_mul(mqr[:, a], qTr[:, a, ssl], mr)
                            nc.vector.scalar_tensor_tensor(
                                mqr[:, a], qTi[:, a, ssl], min_, mqr[:, a],
                                op0=OP.mult, op1=OP.add)
                            nc.vector.tensor_scalar_mul(mqi[:, a], qTr[:, a, ssl], mi)
                            nc.vector.scalar_tensor_tensor(
                                mqi[:, a], qTi[:, a, ssl], mr, mqi[:, a],
                                op0=OP.mult, op1=OP.add)
                            nc.scalar.activation(mqin[:, a], mqi[:, a], AF.Copy,
                                                 scale=-1.0)

                        Er = ep.tile([P, NT, SW], HF, tag="Er")
                        Ei = ep.tile([P, NT, SW], HF, tag="Ei")
                        for tt in range(NT):
                            tsl = slice(tt * P, (tt + 1) * P)
                            pr = psc.tile([P, SW], FP, tag="pr")
                            pi = psc.tile([P, SW], FP, tag="pi")
                            for d in range(NDK):
                                st, sp = d == 0, d == NDK - 1
                                # srT = kTr.mqr - kTi.mqi ; siT = kTi.mqr + kTr.mqi
                                nc.tensor.matmul(pr, kTr[:, d, tsl], mqr[:, d],
                                                 start=st, stop=False)
                                nc.tensor.matmul(pr, kTi[:, d, tsl], mqin[:, d],
                                                 start=False, stop=sp)
                                nc.tensor.matmul(pi, kTi[:, d, tsl], mqr[:, d],
                                                 start=st, stop=False)
                                nc.tensor.matmul(pi, kTr[:, d, tsl], mqi[:, d],
                                                 start=False, stop=sp)
                            # E = exp(s/8 + cb/8), bias indexed by key pos (partition)
                            cbc = tt * 2 * HPC
                            nc.scalar.activation(
                                Er[:, tt], pr, AF.Exp,
                                bias=cb8[:, cbc + h:cbc + h + 1], scale=0.125)
                            nc.scalar.activation(
                                Ei[:, tt], pi, AF.Exp,
                                bias=cb8[:, cbc + HPC + h:cbc + HPC + h + 1],
                                scale=0.125)

                        # context: for each 128-row block of queries
                        for sj in range(SW // P):
                            st_idx = ss * (SW // P) + sj
                            qsl = slice(sj * P, (sj + 1) * P)
                            pcA = pctx.tile([P, 2 * DH + 1], FP, tag="pcA")
                            pcB = pctx.tile([P, 2 * DH + 1], FP, tag="pcB")
                            for tt in range(NT):
                                st, sp = tt == 0, tt == NT - 1
                                nc.tensor.matmul(pcA, Er[:, tt, qsl], vaug[h][:, tt],
                                                 start=st, stop=sp)
                                nc.tensor.matmul(pcB, Ei[:, tt, qsl], vaug[h][:, tt],
                                                 start=st, stop=sp)
                            rr = ctxs.tile([P, 1], FP, tag="rr")
                            ri = ctxs.tile([P, 1], FP, tag="ri")
                            nc.vector.reciprocal(rr, pcA[:, 2 * DH:2 * DH + 1])
                            nc.vector.reciprocal(ri, pcB[:, 2 * DH:2 * DH + 1])
                            # cr = A/sumr - Bvi/sumi ; ci = Avi/sumr + Bvr/sumi
                            tb = ctxs.tile([P, DH], FP, tag="tb")
                            td = ctxs.tile([P, DH], FP, tag="td")
                            cr = ctxs.tile([P, DH], FP, tag="cr")
                            ci = ctxs.tile([P, DH], FP, tag="ci")
                            nc.vector.tensor_scalar_mul(tb, pcB[:, DH:2 * DH], ri)
                            nc.vector.scalar_tensor_tensor(
                                cr, pcA[:, 0:DH], rr, tb, op0=OP.mult, op1=OP.subtract)
                            nc.vector.tensor_scalar_mul(td, pcB[:, 0:DH], ri)
                            nc.vector.scalar_tensor_tensor(
                                ci, pcA[:, DH:2 * DH], rr, td, op0=OP.mult, op1=OP.add)
                            nc.sync.dma_start(out_d[0, h, st_idx], cr)
                            nc.sync.dma_start(out_d[1, h, st_idx], ci)

    _split_multi_waits(nc)
    return nc


def _prep_core_inputs(inputs, core):
    b = core // (N_CORES // B)
    hb = core % (N_CORES // B)
    heads = list(range(hb * HPC, (hb + 1) * HPC))
    cols = slice(hb * VC, (hb + 1) * VC)

    f32 = lambda x: np.ascontiguousarray(np.asarray(x, dtype=np.float32))
    c_f32 = lambda x: np.ascontiguousarray(np.asarray(x, dtype=np.float32))
    hr = f32(inputs["hidden_r"][b]).T    # [D, S]
    hi = f32(inputs["hidden_i"][b]).T

    wv = np.concatenate(
        [f32(inputs["Wv_r"])[:, cols], f32(inputs["Wv_i"])[:, cols],
         f32(inputs["Wcb_r"])[:, heads], f32(inputs["Wcb_i"])[:, heads]], axis=1)
    bv = np.concatenate(
        [np.ones(P, np.float32),
         f32(inputs["bv_r"])[cols], f32(inputs["bv_i"])[cols],
         np.zeros(2 * HPC, np.float32)])

    mr = f32(inputs["mix_r"])[heads]     # [HPC, DK]
    mi = f32(inputs["mix_i"])[heads]
    mixv = np.stack([mr, mi, -mi], axis=-1)  # [HPC, DK, 3]

    c = np.ascontiguousarray
    return {
        "hTr": c_f32(hr.reshape(ND, P, S)),
        "hTi": c_f32(hi.reshape(ND, P, S)),
        "hTin": c_f32((-hi).reshape(ND, P, S)),
        "wqr": c_f32(f32(inputs["Wq_r"]).reshape(ND, P, DK)),
        "wqi": c_f32(f32(inputs["Wq_i"]).reshape(ND, P, DK)),
        "wkr": c_f32(f32(inputs["Wk_r"]).reshape(ND, P, DK)),
        "wki": c_f32(f32(inputs["Wk_i"]).reshape(ND, P, DK)),
        "wvcb": c_f32(wv.reshape(ND, P, WVCB)),
        "bvec": c_f32(bv.reshape(1, P + WVCB)),
        "mixv": c(mixv.reshape(HPC, ND, P, 3).transpose(0, 1, 3, 2)),
    }


def kernel(**inputs):
    global _compiled, LAST_RESULTS
    if _compiled is None:
        _compiled = _build()
    nc = _compiled

    in_maps = [_prep_core_inputs(inputs, c) for c in range(N_CORES)]
    res = run_bass_kernel_spmd(nc, in_maps, core_ids=list(range(N_CORES)),
                               trace=TRACE)
    LAST_RESULTS = res

    out = np.zeros((2, B, S, DV), np.float32)
    for core in range(N_CORES):
        b = core // (N_CORES // B)
        hb = core % (N_CORES // B)
        oc = res.results[core]["out"]  # [2, HPC, NT, P, DH]
        for j in range(HPC):
            h = hb * HPC + j
            out[:, b, :, h * DH:(h + 1) * DH] = oc[:, j].reshape(2, S, DH)
    return out



# revision 5
# speedup vs baseline: 1.2931x; 1.2931x over previous
"""CollaborativeAttention (complex-valued, per-head mixed queries) on 8 trn2 cores.

Sharding: B*H = 24 (batch, head) units -> 3 heads per core.
  core c: batch b = c // 4, head block hb = c % 4 -> heads [3*hb, 3*hb+2].
Each core computes q/k projections for its batch (replicated within the
4-core batch group), v/cb projections for its head block only, then
scores+softmax+context for its 3 heads.

v2 (Karatsuba): every complex matmul (q/k projections, scores) uses the
3-mult form m1=ar@br, m2=ai@bi, m3=(ar+ai)@(br+bi); real=m1-m2,
imag=m3-m1-m2 -- 18 PE matmuls per tile-group instead of 24 (-25% on the
two dominant GEMM phases).  Host ships h_sum = hr+hi and packed
[Wr|Wi|Wr+Wi] weight tiles so no negated copies and no extra DVE adds are
needed on the contraction side; each weight byte is DMA'd exactly once
(14.2 MB vs 37.7 MB for the ss/ntb-refetching v1).  Hidden states stay
SBUF-resident for the whole projection phase (per-d-tile DMAs so the
first matmuls start ~2 us in).  q/k/mixed-query tensors are stored fp16:
the score matmuls run fp16 (same 1 cyc/row PE rate as fp32r, ~5e-4
relative logit error), SBUF drops well under budget, and the per-head
mixing runs in the DVE 2x 16-bit mode.  Scores accumulate per key-tile in
3 PSUM banks (double-buffered = 6) and are combined with one ScalarE
stage copy + 3 DVE subs, then exp'd with the content bias fused as a
per-partition ACT bias exactly as v1.  Mixing for block n+1 is emitted
before block n's context matmuls so the PE never waits on the DVE at
block boundaries; block 0's mixing is interleaved into the q-projection
combines.

Layout notes (unchanged from v1):
 - hidden is transposed on the HOST (fp32 has no DMA transpose).
 - scores are computed transposed, sT[t, s] (t = key pos on partitions), so
   the content bias (indexed by t) is a per-partition ACT bias fused into the
   exp, and probs land directly in the lhsT layout the context matmul wants.
 - softmax denominators come from a ones-column appended to [vr | vi] in the
   context matmul's moving operand; normalization happens on the tiny
   [128, 64] context tiles instead of the [1024, 1024] prob matrices.
This walrus build encodes at most one sync-wait per instruction, so a
post-pass (_split_multi_waits) peels extra waits onto NoOps.
"""

import sys

for _p in ("/opt/trn_rl_repo", "/root/.axon_site", "/root/.axon_site/_ro/trn_rl_repo",
           "/root/.axon_site/_ro/pypackages"):
    if _p not in sys.path:
        sys.path.append(_p)

import numpy as np

import concourse.bass as bass
import concourse.mybir as mybir
import concourse.tile as tile
from concourse.bass_utils import run_bass_kernel_spmd

B, S, D, H = 2, 1024, 768, 12
DK = DV = 768
DH = DV // H          # 64 per-head value dim
HPC = 3               # heads per core
N_CORES = 8
P = 128
ND = D // P           # 6 d-tiles (contraction)
NDK = DK // P         # 6 q/k n-tiles
NT = S // P           # 8 token tiles
SW = 512              # s-slice width for scores/projections
NS = S // SW          # 2 s-slices
VC = HPC * DH         # 192 value cols per core
WVCB = 2 * VC + 2 * HPC  # 390: [Wv_r | Wv_i | Wcb_r | Wcb_i] cols

FP = mybir.dt.float32
FR = mybir.dt.float32r
HF = mybir.dt.float16
AF = mybir.ActivationFunctionType
OP = mybir.AluOpType

TRACE = False
LAST_RESULTS = None

_compiled = None


def _split_multi_waits(nc):
    """The walrus build here encodes at most ONE sync-wait per instruction
    ("Too many sync wait commands" in setupSyncWait otherwise). Tile freely
    emits several. Split the extras onto single-wait NoOps that precede the
    instruction in the same engine stream."""
    for fn in nc.m.functions:
        for bb in fn.blocks:
            out = []
            for ins in bb.instructions:
                si = ins.sync_info
                if si is not None and len(si.on_wait) > 1:
                    waits = list(si.on_wait)
                    for j, w in enumerate(waits[:-1]):
                        nop = mybir.InstNoOp(name=f"{ins.name}-ws{j}",
                                             ins=[], outs=[])
                        nop.engine = ins.engine
                        nop.sync_info = mybir.SyncInfo(on_wait=[w], on_update=[])
                        out.append(nop)
                    ins.sync_info = mybir.SyncInfo(on_wait=[waits[-1]],
                                                   on_update=list(si.on_update))
                out.append(ins)
            bb.instructions = out


def _build(split_waits=True):
    """Build the SPMD Bass program (identical on all 8 cores)."""
    nc = bass.Bass(trn_type="TRN2")

    hTr_d = nc.dram_tensor("hTr", [ND, P, S], FR, kind="ExternalInput")
    hTi_d = nc.dram_tensor("hTi", [ND, P, S], FR, kind="ExternalInput")
    hTs_d = nc.dram_tensor("hTs", [ND, P, S], FR, kind="ExternalInput")
    # packed [Wr | Wi | Wr+Wi] per (proj: 0=k 1=q, out n-tile, contraction d-tile)
    wpk_d = nc.dram_tensor("wpk", [2, NDK, ND, P, 3 * P], FR,
                           kind="ExternalInput")
    wvcb_d = nc.dram_tensor("wvcb", [ND, P, WVCB], FR, kind="ExternalInput")
    bvec_d = nc.dram_tensor("bvec", [1, P + WVCB], FR, kind="ExternalInput")
    mixv_d = nc.dram_tensor("mixv", [HPC, NDK, 3, P], FP, kind="ExternalInput")
    out_d = nc.dram_tensor("out", [2, HPC, NT, P, DH], FP, kind="ExternalOutput")

    with tile.TileContext(nc) as tc:
        with (
            tc.tile_pool(name="persist", bufs=1) as persist,
            tc.tile_pool(name="vstuff", bufs=1) as vstuff,
            tc.tile_pool(name="mqp", bufs=1) as mqp,
        ):
            # ---- persistent tensors -------------------------------------
            qTr = persist.tile([P, NDK, S], HF)
            qTi = persist.tile([P, NDK, S], HF)
            kTr = persist.tile([P, NDK, S], HF)
            kTi = persist.tile([P, NDK, S], HF)
            kTs = persist.tile([P, NDK, S], HF)

            bvec_sb = vstuff.tile([1, P + WVCB], FR)
            nc.sync.dma_start(bvec_sb, bvec_d[:])
            mixv = vstuff.tile([P, HPC * NDK * 3], FP)
            nc.sync.dma_start(mixv, mixv_d[:].rearrange("h a c p -> p (h a c)"))
            wvcb_sb = vstuff.tile([P, ND, WVCB], FR)
            nc.sync.dma_start(wvcb_sb, wvcb_d[:].rearrange("a p c -> p a c"))
            # per-head context rhs: [vr_h | vi_h | 1]
            vaug = [vstuff.tile([P, NT, 2 * DH + 1], HF, tag=f"vaug{h}",
                                name=f"vaug{h}")
                    for h in range(HPC)]
            for h in range(HPC):
                nc.vector.memset(vaug[h][:, :, 2 * DH], 1.0)
            # (cbr/8 | cbi/8) per head, flattened: col = tt*2*HPC + (0|HPC) + h
            cb8 = vstuff.tile([P, NT * 2 * HPC], FP)

            def emit_mix(h, ss, a, mq):
                """mixed query for head h, slice ss, n-tile a (fp16, DVE 2x)."""
                mqr, mqi, mqs = mq
                ssl = slice(ss * SW, (ss + 1) * SW)
                mbase = (h * NDK + a) * 3
                mr = mixv[:, mbase:mbase + 1]
                mi = mixv[:, mbase + 1:mbase + 2]
                min_ = mixv[:, mbase + 2:mbase + 3]
                # mqr = qTr*mr - qTi*mi ; mqi = qTr*mi + qTi*mr ; mqs = mqr+mqi
                nc.vector.tensor_scalar_mul(mqr[:, a], qTr[:, a, ssl], mr)
                nc.vector.scalar_tensor_tensor(
                    mqr[:, a], qTi[:, a, ssl], min_, mqr[:, a],
                    op0=OP.mult, op1=OP.add)
                nc.vector.tensor_scalar_mul(mqi[:, a], qTr[:, a, ssl], mi)
                nc.vector.scalar_tensor_tensor(
                    mqi[:, a], qTi[:, a, ssl], mr, mqi[:, a],
                    op0=OP.mult, op1=OP.add)
                nc.vector.tensor_add(mqs[:, a], mqr[:, a], mqi[:, a])

            def alloc_mq():
                return (mqp.tile([P, NDK, SW], HF, tag="mqr", name="mqr"),
                        mqp.tile([P, NDK, SW], HF, tag="mqi", name="mqi"),
                        mqp.tile([P, NDK, SW], HF, tag="mqs", name="mqs"))

            # ---- phase P: projections -----------------------------------
            with (
                tc.tile_pool(name="hload", bufs=1) as hload,
                tc.tile_pool(name="wstream", bufs=2) as wstream,
                tc.tile_pool(name="pproj", bufs=1, space="PSUM") as pproj,
                tc.tile_pool(name="pv", bufs=1, space="PSUM") as pv,
                tc.tile_pool(name="cbtmp", bufs=2) as cbtmp,
                tc.tile_pool(name="vstage", bufs=2) as vstage,
                tc.tile_pool(name="qkstage", bufs=2) as qkstage,
            ):
                # full-S resident hidden; per-d DMAs so compute starts early
                hr = hload.tile([P, ND, S], FR, tag="hr")
                hi = hload.tile([P, ND, S], FR, tag="hi")
                hs = hload.tile([P, ND, S], FR, tag="hs")
                wk0 = wstream.tile([P, ND, 3 * P], FR, tag="w")
                nc.sync.dma_start(wk0, wpk_d[0, 0].rearrange("a p c -> p a c"))
                for d in range(ND):
                    nc.sync.dma_start(hr[:, d], hTr_d[d])
                for d in range(ND):
                    nc.sync.dma_start(hi[:, d], hTi_d[d])
                for d in range(ND):
                    nc.sync.dma_start(hs[:, d], hTs_d[d])

                # -- v / cb projections: [tok, c] = sum_d hT[d, tok] @ Wbig[d, c]
                for tt in range(NT):
                    tsl = slice(tt * P, (tt + 1) * P)
                    psA = pv.tile([P, WVCB], FP, tag="psA")
                    psB = pv.tile([P, WVCB], FP, tag="psB")
                    # bias row (bv | 0) into psA first (fewest deps first)
                    nc.tensor.matmul(psA, bvec_sb[:, :P], bvec_sb[:, P:],
                                     start=True, stop=False)
                    for d in range(ND):
                        nc.tensor.matmul(psA, hr[:, d, tsl], wvcb_sb[:, d],
                                         start=False, stop=(d == ND - 1))
                    for d in range(ND):
                        nc.tensor.matmul(psB, hi[:, d, tsl], wvcb_sb[:, d],
                                         start=(d == 0), stop=(d == ND - 1))
                    # DVE can read only one PSUM operand; stage A in SBUF
                    sA = vstage.tile([P, WVCB], FP, tag="sA")
                    nc.scalar.activation(sA, psA, AF.Copy)
                    for h in range(HPC):
                        c0 = h * DH
                        # vr_h = A[vr] - B[vi];  vi_h = A[vi] + B[vr]
                        nc.vector.tensor_sub(vaug[h][:, tt, 0:DH],
                                             sA[:, c0:c0 + DH],
                                             psB[:, VC + c0:VC + c0 + DH])
                        nc.vector.tensor_add(vaug[h][:, tt, DH:2 * DH],
                                             sA[:, VC + c0:VC + c0 + DH],
                                             psB[:, c0:c0 + DH])
                    # cb8: (A[cbr] - B[cbi])/8 , (A[cbi] + B[cbr])/8
                    tr = cbtmp.tile([P, HPC], FP, tag="tr")
                    ti = cbtmp.tile([P, HPC], FP, tag="ti")
                    nc.vector.tensor_sub(tr, sA[:, 2 * VC:2 * VC + HPC],
                                         psB[:, 2 * VC + HPC:2 * VC + 2 * HPC])
                    nc.vector.tensor_add(ti, sA[:, 2 * VC + HPC:2 * VC + 2 * HPC],
                                         psB[:, 2 * VC:2 * VC + HPC])
                    cbc = tt * 2 * HPC
                    nc.vector.tensor_scalar_mul(cb8[:, cbc:cbc + HPC], tr, 0.125)
                    nc.vector.tensor_scalar_mul(
                        cb8[:, cbc + HPC:cbc + 2 * HPC], ti, 0.125)

                # -- q/k projections, Karatsuba: per (proj, nt, ss)
                #    m1 = Wr.hr, m2 = Wi.hi, m3 = (Wr+Wi).(hr+hi)
                #    real = m1-m2, imag = m3-m1-m2, ksum = m3-2*m2
                mq0 = alloc_mq()  # block (h=0, ss=0) mixing, emitted in q loop
                for pi, (dst_r, dst_i) in enumerate(((kTr, kTi), (qTr, qTi))):
                    is_k = pi == 0
                    for nt in range(NDK):
                        if pi == 0 and nt == 0:
                            w = wk0
                        else:
                            w = wstream.tile([P, ND, 3 * P], FR, tag="w")
                            nc.sync.dma_start(
                                w, wpk_d[pi, nt].rearrange("a p c -> p a c"))
                        ps = {}
                        for ss in range(NS):
                            for j in range(3):
                                ps[ss, j] = pproj.tile([P, SW], FP,
                                                       tag=f"pp{ss}{j}",
                                                       name=f"pp{nt}_{ss}{j}")
                        # m1/m2 for both ss first: m3 needs the late-arriving
                        # hsum stream at startup
                        for ss in range(NS):
                            ssl = slice(ss * SW, (ss + 1) * SW)
                            for d in range(ND):
                                nc.tensor.matmul(ps[ss, 0], w[:, d, 0:P],
                                                 hr[:, d, ssl],
                                                 start=(d == 0), stop=(d == ND - 1))
                            for d in range(ND):
                                nc.tensor.matmul(ps[ss, 1], w[:, d, P:2 * P],
                                                 hi[:, d, ssl],
                                                 start=(d == 0), stop=(d == ND - 1))
                        for ss in range(NS):
                            ssl = slice(ss * SW, (ss + 1) * SW)
                            for d in range(ND):
                                nc.tensor.matmul(ps[ss, 2], w[:, d, 2 * P:3 * P],
                                                 hs[:, d, ssl],
                                                 start=(d == 0), stop=(d == ND - 1))
                        for ss in range(NS):
                            ssl = slice(ss * SW, (ss + 1) * SW)
                            m1, m2, m3 = ps[ss, 0], ps[ss, 1], ps[ss, 2]
                            s2 = qkstage.tile([P, SW], FP, tag="s2")
                            t3 = qkstage.tile([P, SW], FP, tag="t3")
                            nc.scalar.activation(s2, m2, AF.Copy)
                            nc.vector.tensor_sub(dst_r[:, nt, ssl], m1, s2)
                            nc.vector.tensor_sub(t3, m3, s2)
                            nc.vector.tensor_sub(dst_i[:, nt, ssl], t3, m1)
                            if is_k:
                                nc.vector.scalar_tensor_tensor(
                                    kTs[:, nt, ssl], s2, -2.0, m3,
                                    op0=OP.mult, op1=OP.add)
                            elif ss == 0:
                                emit_mix(0, 0, nt, mq0)

            # ---- phase S: per-head scores -> softmax -> context ---------
            blocks = [(h, ss) for h in range(HPC) for ss in range(NS)]
            block_mq = {blocks[0]: mq0}
            with (
                tc.tile_pool(name="ep", bufs=1) as ep,
                tc.tile_pool(name="sstage", bufs=2) as sstage,
                tc.tile_pool(name="psc", bufs=1, space="PSUM") as psc,
                tc.tile_pool(name="pctx", bufs=1, space="PSUM") as pctx,
                tc.tile_pool(name="ctxs", bufs=4) as ctxs,
            ):
                for bi, (h, ss) in enumerate(blocks):
                    ssl = slice(ss * SW, (ss + 1) * SW)
                    if (h, ss) not in block_mq:
                        # emitted at the end of the previous block's tt loop
                        raise AssertionError("mixing not pre-emitted")
                    mqr, mqi, mqs = block_mq[h, ss]

                    Er = ep.tile([P, NT, SW], HF, tag="Er")
                    Ei = ep.tile([P, NT, SW], HF, tag="Ei")
                    for tt in range(NT):
                        tsl = slice(tt * P, (tt + 1) * P)
                        m1 = psc.tile([P, SW], FP, tag=f"m1{tt % 2}")
                        m2 = psc.tile([P, SW], FP, tag=f"m2{tt % 2}")
                        m3 = psc.tile([P, SW], FP, tag=f"m3{tt % 2}")
                        for d in range(NDK):
                            nc.tensor.matmul(m1, kTr[:, d, tsl], mqr[:, d],
                                             start=(d == 0), stop=(d == NDK - 1))
                        for d in range(NDK):
                            nc.tensor.matmul(m2, kTi[:, d, tsl], mqi[:, d],
                                             start=(d == 0), stop=(d == NDK - 1))
                        for d in range(NDK):
                            nc.tensor.matmul(m3, kTs[:, d, tsl], mqs[:, d],
                                             start=(d == 0), stop=(d == NDK - 1))
                        # sr = m1-m2, si = m3-m1-m2;  E = exp(s/8 + cb/8)
                        s2 = sstage.tile([P, SW], FP, tag="s2")
                        tr = sstage.tile([P, SW], FP, tag="tr")
                        ti = sstage.tile([P, SW], FP, tag="ti")
                        nc.scalar.activation(s2, m2, AF.Copy)
                        nc.vector.tensor_sub(tr, m1, s2)
                        cbc = tt * 2 * HPC
                        nc.scalar.activation(
                            Er[:, tt], tr, AF.Exp,
                            bias=cb8[:, cbc + h:cbc + h + 1], scale=0.125)
                        nc.vector.tensor_sub(ti, m3, s2)
                        nc.vector.tensor_sub(ti, ti, m1)
                        nc.scalar.activation(
                            Ei[:, tt], ti, AF.Exp,
                            bias=cb8[:, cbc + HPC + h:cbc + HPC + h + 1],
                            scale=0.125)

                    # mixing for the next block: runs on the DVE during this
                    # block's context matmuls, so the PE never waits for it
                    if bi + 1 < len(blocks):
                        nh, nss = blocks[bi + 1]
                        nmq = alloc_mq()
                        block_mq[nh, nss] = nmq
                        for a in range(NDK):
                            emit_mix(nh, nss, a, nmq)

                    # context: for each 128-row block of queries
                    for sj in range(SW // P):
                        st_idx = ss * (SW // P) + sj
                        qsl = slice(sj * P, (sj + 1) * P)
                        pcA = pctx.tile([P, 2 * DH + 1], FP, tag="pcA")
                        pcB = pctx.tile([P, 2 * DH + 1], FP, tag="pcB")
                        for tt in range(NT):
                            st, sp = tt == 0, tt == NT - 1
                            nc.tensor.matmul(pcA, Er[:, tt, qsl], vaug[h][:, tt],
                                             start=st, stop=sp)
                            nc.tensor.matmul(pcB, Ei[:, tt, qsl], vaug[h][:, tt],
                                             start=st, stop=sp)
                        rr = ctxs.tile([P, 1], FP, tag="rr")
                        ri = ctxs.tile([P, 1], FP, tag="ri")
                        nc.vector.reciprocal(rr, pcA[:, 2 * DH:2 * DH + 1])
                        nc.vector.reciprocal(ri, pcB[:, 2 * DH:2 * DH + 1])
                        # cr = A/sumr - Bvi/sumi ; ci = Avi/sumr + Bvr/sumi
                        tb = ctxs.tile([P, DH], FP, tag="tb")
                        td = ctxs.tile([P, DH], FP, tag="td")
                        cr = ctxs.tile([P, DH], FP, tag="cr")
                        ci = ctxs.tile([P, DH], FP, tag="ci")
                        nc.vector.tensor_scalar_mul(tb, pcB[:, DH:2 * DH], ri)
                        nc.vector.scalar_tensor_tensor(
                            cr, pcA[:, 0:DH], rr, tb, op0=OP.mult, op1=OP.subtract)
                        nc.vector.tensor_scalar_mul(td, pcB[:, 0:DH], ri)
                        nc.vector.scalar_tensor_tensor(
                            ci, pcA[:, DH:2 * DH], rr, td, op0=OP.mult, op1=OP.add)
                        nc.sync.dma_start(out_d[0, h, st_idx], cr)
                        nc.sync.dma_start(out_d[1, h, st_idx], ci)

    if split_waits:
        _split_multi_waits(nc)
    return nc


_shared_prep = None


def _prep_shared(inputs):
    """Core-independent packed tensors (weights are replicated)."""
    f32 = lambda x: np.asarray(x, dtype=np.float32)
    c = np.ascontiguousarray

    wpk = np.empty((2, NDK, ND, P, 3 * P), np.float32)
    for pi, (wr_name, wi_name) in enumerate((("Wk_r", "Wk_i"), ("Wq_r", "Wq_i"))):
        wr, wi = f32(inputs[wr_name]), f32(inputs[wi_name])
        W3 = np.stack([wr, wi, wr + wi])              # [3, D, DK]
        wpk[pi] = (W3.reshape(3, ND, P, NDK, P)
                   .transpose(3, 1, 2, 0, 4).reshape(NDK, ND, P, 3 * P))
    hT = {}
    for b in range(B):
        hr = f32(inputs["hidden_r"][b]).T             # [D, S]
        hi = f32(inputs["hidden_i"][b]).T
        hT[b] = (c(hr.reshape(ND, P, S)), c(hi.reshape(ND, P, S)),
                 c((hr + hi).reshape(ND, P, S)))
    return c(wpk), hT


def _prep_core_inputs(inputs, core, wpk, hT):
    hb = core % (N_CORES // B)
    heads = list(range(hb * HPC, (hb + 1) * HPC))
    cols = slice(hb * VC, (hb + 1) * VC)

    f32 = lambda x: np.asarray(x, dtype=np.float32)
    c = np.ascontiguousarray
    hr, hi, hs = hT[core // (N_CORES // B)]

    wv = np.concatenate(
        [f32(inputs["Wv_r"])[:, cols], f32(inputs["Wv_i"])[:, cols],
         f32(inputs["Wcb_r"])[:, heads], f32(inputs["Wcb_i"])[:, heads]], axis=1)
    bv = np.concatenate(
        [np.ones(P, np.float32),
         f32(inputs["bv_r"])[cols], f32(inputs["bv_i"])[cols],
         np.zeros(2 * HPC, np.float32)])

    mr = f32(inputs["mix_r"])[heads]     # [HPC, DK]
    mi = f32(inputs["mix_i"])[heads]
    mixv = np.stack([mr, mi, -mi], axis=-1)  # [HPC, DK, 3]

    return {
        "hTr": hr,
        "hTi": hi,
        "hTs": hs,
        "wpk": wpk,
        "wvcb": c(wv.reshape(ND, P, WVCB)),
        "bvec": c(bv.reshape(1, P + WVCB)),
        "mixv": c(mixv.reshape(HPC, NDK, P, 3).transpose(0, 1, 3, 2)),
    }


def kernel(**inputs):
    global _compiled, LAST_RESULTS
    if _compiled is None:
        _compiled = _build()
    nc = _compiled

    wpk, hT = _prep_shared(inputs)
    in_maps = [_prep_core_inputs(inputs, c, wpk, hT) for c in range(N_CORES)]
    res = run_bass_kernel_spmd(nc, in_maps, core_ids=list(range(N_CORES)),
                               trace=TRACE)
    LAST_RESULTS = res

    out = np.zeros((2, B, S, DV), np.float32)
    for core in range(N_CORES):
        b = core // (N_CORES // B)
        hb = core % (N_CORES // B)
        oc = res.results[core]["out"]  # [2, HPC, NT, P, DH]
        for j in range(HPC):
            h = hb * HPC + j
            out[:, b, :, h * DH:(h + 1) * DH] = oc[:, j].reshape(2, S, DH)
    return out
